# revision 1
# baseline (speedup 1.0000x reference)
"""Trainium2 Bass kernel for single-head causal attention.

  q = Xq @ Wq.T + bq ; k = Xk @ Wk.T + bk ; v = Xv @ Wv.T + bv
  out = softmax((q k^T + causal_mask)/sqrt(D)) @ v

Shapes: B=4, S=2048, D=1024, fp32.  8 NeuronCores, SPMD.

Sharding (uniform causal schedule -> identical program on every core):
  core c handles batch b = c//2, parity h = c%2.
  Within the batch, S splits into 16 q-tiles of 128.  q-tile g needs
  ceil((g+1)/4) k-chunks of 512.  Core (b, h) takes, for every chunk-count
  class n in {1,2,3,4}, the two tiles g = 4*(n-1) + 2*h and +2*h+1.  So each
  core owns 8 q-tiles with chunk counts [1,1,2,2,3,3,4,4] - the same static
  schedule everywhere; only the DATA (inputs, masks) differs per core.

All matmuls run as float32r (fp32 storage, FP22 multiply, fp32 accumulate,
1 PE row/cycle at free-dim >= 256).  Attention is k-chunk-major so the
KT / V projections stream through DRAM scratch once per chunk.
"""

from contextlib import ExitStack

import numpy as np

import concourse.bacc as bacc
import concourse.mybir as mybir
import concourse.tile as tile
from concourse.bass_utils import run_bass_kernel_spmd
from concourse.masks import make_identity

P = 128
D = 1024
S = 2048
B = 4
N_CORES = 8
EO = D // P            # 8 contraction chunks
DO = D // P            # 8 output-dim chunks
KC = S // 512          # 4 k-chunks of 512
NQ = 8                 # q-slots per core
# slot j -> (n_chunks, pair_idx); global q-tile g = 4*(n-1) + 2*h + p
SLOTS = [(1, 0), (1, 1), (2, 0), (2, 1), (3, 0), (3, 1), (4, 0), (4, 1)]
F32 = mybir.dt.float32
F32R = mybir.dt.float32r
NEG = -1.0e9

_PROG_CACHE = {}


def _slot_gtiles(h, causal):
    if causal:
        return [4 * (n - 1) + 2 * h + p for (n, p) in SLOTS]
    return [8 * h + j for j in range(NQ)]


def build_program(causal: bool):
    nc = bacc.Bacc(trn_type="TRN2", target_bir_lowering=False, debug=False)

    def din(name, shape, dt=F32):
        return nc.dram_tensor(name, shape, dt, kind="ExternalInput").ap()

    xq = din("xq", [2, P, EO, 512], F32R)
    xk = din("xk", [KC, P, EO, 512], F32R)
    xv = din("xv", [16, P, EO, P], F32R)
    wq = din("wq", [P, EO, D], F32R)
    wk = din("wk", [P, EO, D], F32R)
    wv = din("wv", [P, EO, D], F32R)
    bq = din("bq", [P, DO])
    bk = din("bk", [P, DO])
    bv = din("bv", [P, D])
    cm = din("cm", [P, 2, 512])
    out = nc.dram_tensor("out", [NQ, P, D], F32, kind="ExternalOutput").ap()
    ktd = nc.dram_tensor("ktd", [KC, P, DO, 512], F32R).ap()
    vd = nc.dram_tensor("vd", [16, P, D], F32R).ap()

    Ident = mybir.ActivationFunctionType.Identity
    Exp = mybir.ActivationFunctionType.Exp
    add = mybir.AluOpType.add
    mult = mybir.AluOpType.mult

    nchunks = [n for (n, _) in SLOTS] if causal else [KC] * NQ

    with tile.TileContext(nc, pool_alloc_mode="queue") as tc, ExitStack() as top:
        const = top.enter_context(tc.tile_pool(name="const", bufs=1))
        ident = const.tile([P, P], F32)
        make_identity(nc, ident)
        bq_sb = const.tile([P, DO], F32)
        nc.gpsimd.dma_start(out=bq_sb, in_=bq)
        bk_sb = const.tile([P, DO], F32)
        nc.gpsimd.dma_start(out=bk_sb, in_=bk)
        bv_sb = const.tile([P, D], F32)
        nc.gpsimd.dma_start(out=bv_sb, in_=bv)
        cm_sb = const.tile([P, 2, 512], F32)
        nc.gpsimd.dma_start(out=cm_sb, in_=cm)

        qtp = top.enter_context(tc.tile_pool(name="qtp", bufs=1))
        qt_sb = qtp.tile([P, DO, D], F32R)
        pf = top.enter_context(tc.tile_pool(name="pf", bufs=1))
        kt0_sb = pf.tile([P, DO, 512], F32R, name="kt0_sb")
        v0_sb = pf.tile([P, KC, D], F32R, name="v0_sb")

        # ---------------- projections ----------------
        with tc.tile_pool(name="wt", bufs=2) as wtp, \
             tc.tile_pool(name="xin", bufs=3) as xinp, \
             tc.tile_pool(name="xvp", bufs=2) as xvp, \
             tc.tile_pool(name="stg", bufs=3) as stg, \
             tc.tile_pool(name="stgv", bufs=2) as stgv, \
             tc.tile_pool(name="psA", bufs=3, space="PSUM") as psA, \
             tc.tile_pool(name="psB", bufs=2, space="PSUM") as psB:

            # K projection -> ktd (transposed layout [d, k]), bias folded in
            wk_sb = wtp.tile([P, EO, D], F32R, tag="wt", name="wk_sb")
            for kc in range(KC):
                xk_t = xinp.tile([P, EO, 512], F32R, tag="xin", name=f"xk_t{kc}")
                for eo in range(EO):
                    eng = nc.sync if eo % 2 == 0 else nc.scalar
                    if kc == 0:
                        eng.dma_start(out=wk_sb[:, eo, :], in_=wk[:, eo, :])
                    eng.dma_start(out=xk_t[:, eo, :], in_=xk[kc, :, eo, :])
                for do in range(DO):
                    ps = psA.tile([P, 512], F32, tag="psA", name=f"psk{kc}_{do}")
                    for eo in range(EO):
                        nc.tensor.matmul(
                            ps,
                            lhsT=wk_sb[:, eo, do * P:(do + 1) * P],
                            rhs=xk_t[:, eo, :],
                            start=(eo == 0), stop=(eo == EO - 1))
                    kst = stg.tile([P, 512], F32R, tag="stg", name=f"kst{kc}_{do}")
                    nc.scalar.activation(out=kst, in_=ps, func=Ident,
                                         bias=bk_sb[:, do:do + 1])
                    nc.gpsimd.dma_start(out=ktd[kc, :, do, :], in_=kst)

            nc.sync.dma_start(out=kt0_sb, in_=ktd[0])

            # Q projection -> qt_sb resident (transposed layout [d, q])
            wq_sb = wtp.tile([P, EO, D], F32R, tag="wt", name="wq_sb")
            for sc in range(2):
                xq_t = xinp.tile([P, EO, 512], F32R, tag="xin", name=f"xq_t{sc}")
                for eo in range(EO):
                    eng = nc.sync if eo % 2 == 0 else nc.scalar
                    if sc == 0:
                        eng.dma_start(out=wq_sb[:, eo, :], in_=wq[:, eo, :])
                    eng.dma_start(out=xq_t[:, eo, :], in_=xq[sc, :, eo, :])
                for do in range(DO):
                    ps = psA.tile([P, 512], F32, tag="psA", name=f"psq{sc}_{do}")
                    for eo in range(EO):
                        nc.tensor.matmul(
                            ps,
                            lhsT=wq_sb[:, eo, do * P:(do + 1) * P],
                            rhs=xq_t[:, eo, :],
                            start=(eo == 0), stop=(eo == EO - 1))
                    nc.scalar.activation(
                        out=qt_sb[:, do, sc * 512:(sc + 1) * 512], in_=ps,
                        func=Ident, bias=bq_sb[:, do:do + 1])

            # V projection -> vd (natural layout [s, d]), bias folded at the end
            wv_sb = wtp.tile([P, EO, D], F32R, tag="wt", name="wv_sb")
            for st in range(16):
                xv_t = xvp.tile([P, EO, P], F32R, tag="xv", name=f"xv_t{st}")
                if st == 0:
                    for eo in range(EO):
                        eng = nc.sync if eo % 2 == 0 else nc.scalar
                        eng.dma_start(out=wv_sb[:, eo, :], in_=wv[:, eo, :])
                eng = nc.sync if st % 2 == 0 else nc.scalar
                eng.dma_start(out=xv_t, in_=xv[st])
                ps2 = psB.tile([P, D], F32, tag="psB", name=f"psv{st}")
                for half in range(2):
                    for eo in range(EO):
                        nc.tensor.matmul(
                            ps2[:, half * 512:(half + 1) * 512],
                            lhsT=xv_t[:, eo, :],
                            rhs=wv_sb[:, eo, half * 512:(half + 1) * 512],
                            start=(eo == 0), stop=(eo == EO - 1))
                vst = stgv.tile([P, D], F32R, tag="stgv", name=f"vst{st}")
                nc.vector.tensor_copy(out=vst, in_=ps2)
                nc.gpsimd.dma_start(out=vd[st], in_=vst)
                if st < 4:
                    nc.sync.dma_start(out=v0_sb[:, st, :], in_=vd[st])

        # ---------------- attention, k-chunk-major ----------------
        with tc.tile_pool(name="ktS", bufs=2) as ktS, \
             tc.tile_pool(name="vS", bufs=2) as vS, \
             tc.tile_pool(name="pp", bufs=3) as pp, \
             tc.tile_pool(name="ptp", bufs=3) as ptp, \
             tc.tile_pool(name="avp", bufs=NQ) as avp, \
             tc.tile_pool(name="accp", bufs=NQ) as accp, \
             tc.tile_pool(name="denp", bufs=4) as denp, \
             tc.tile_pool(name="outp", bufs=3) as outp, \
             tc.tile_pool(name="psS", bufs=2, space="PSUM") as psS, \
             tc.tile_pool(name="psT", bufs=2, space="PSUM") as psT, \
             tc.tile_pool(name="psV", bufs=2, space="PSUM") as psV:

            accs = [accp.tile([P, KC], F32, tag="acc", name=f"acc{j}")
                    for j in range(NQ)]
            avs = [avp.tile([P, D], F32, tag="av", name=f"av{j}")
                   for j in range(NQ)]

            for c in range(KC):
                if c == 0:
                    kt_t, v_t = kt0_sb, v0_sb
                else:
                    kt_t = ktS.tile([P, DO, 512], F32R, tag="kt", name=f"kt_t{c}")
                    nc.sync.dma_start(out=kt_t, in_=ktd[c])
                    v_t = vS.tile([P, KC, D], F32R, tag="v", name=f"v_t{c}")
                    for t in range(4):
                        eng = nc.scalar if t % 2 == 0 else nc.sync
                        eng.dma_start(out=v_t[:, t, :], in_=vd[4 * c + t])

                for j in range(NQ):
                    n_j = nchunks[j]
                    if c >= n_j:
                        continue
                    p_j = SLOTS[j][1]
                    ps = psS.tile([P, 512], F32, tag="s", name=f"ps{c}_{j}")
                    for do in range(DO):
                        nc.tensor.matmul(
                            ps,
                            lhsT=qt_sb[:, do, j * P:(j + 1) * P],
                            rhs=kt_t[:, do, :],
                            start=(do == 0), stop=(do == DO - 1))
                    if causal and c == n_j - 1:
                        nc.vector.tensor_tensor(out=ps, in0=ps,
                                                in1=cm_sb[:, p_j, :], op=add)
                    pe = pp.tile([P, 512], F32, tag="p", name=f"pe{c}_{j}")
                    nc.scalar.activation(out=pe, in_=ps, func=Exp,
                                         scale=float(1.0 / np.sqrt(D)),
                                         accum_out=accs[j][:, c:c + 1])
                    ptps = psT.tile([P, 4, P], F32, tag="t", name=f"ptps{c}_{j}")
                    for t in range(4):
                        nc.tensor.transpose(ptps[:, t, :],
                                            pe[:, t * P:(t + 1) * P], ident)
                    pt = ptp.tile([P, 4, P], F32R, tag="pt", name=f"pt{c}_{j}")
                    nc.vector.tensor_copy(out=pt, in_=ptps)
                    av_ps = psV.tile([P, D], F32, tag="avp", name=f"avps{c}_{j}")
                    for half in range(2):
                        for t in range(4):
                            nc.tensor.matmul(
                                av_ps[:, half * 512:(half + 1) * 512],
                                lhsT=pt[:, t, :],
                                rhs=v_t[:, t, half * 512:(half + 1) * 512],
                                start=(t == 0), stop=(t == 3))
                    if c == 0:
                        nc.vector.tensor_copy(out=avs[j], in_=av_ps)
                    else:
                        nc.vector.tensor_add(avs[j], avs[j], av_ps)

                    if c == n_j - 1:
                        den = denp.tile([P, 1], F32, tag="den", name=f"den{j}")
                        nc.vector.tensor_reduce(
                            out=den, in_=accs[j][:, 0:n_j],
                            axis=mybir.AxisListType.X, op=add)
                        rec = denp.tile([P, 1], F32, tag="rec", name=f"rec{j}")
                        nc.vector.reciprocal(out=rec, in_=den)
                        o = outp.tile([P, D], F32, tag="o", name=f"o{j}")
                        nc.vector.scalar_tensor_tensor(
                            out=o, in0=avs[j], scalar=rec, in1=bv_sb,
                            op0=mult, op1=add)
                        nc.scalar.dma_start(out=out[j], in_=o)

    nc.compile()
    return nc


def _get_program(causal: bool):
    key = bool(causal)
    if key not in _PROG_CACHE:
        _PROG_CACHE[key] = build_program(key)
    return _PROG_CACHE[key]


def _shard_inputs(encoded_q, encoded_k, encoded_v, W_q, b_q, W_k, b_k,
                  W_v, b_v, causal):
    """Build the per-core in_maps (all host-side numpy)."""
    wqh = np.ascontiguousarray(W_q.T.reshape(EO, P, D).transpose(1, 0, 2))
    wkh = np.ascontiguousarray(W_k.T.reshape(EO, P, D).transpose(1, 0, 2))
    wvh = np.ascontiguousarray(W_v.T.reshape(EO, P, D).transpose(1, 0, 2))
    bqh = np.ascontiguousarray(b_q.reshape(DO, P).T)
    bkh = np.ascontiguousarray(b_k.reshape(DO, P).T)
    bvh = np.ascontiguousarray(np.broadcast_to(b_v, (P, D)))

    qi = np.arange(P)[:, None]
    kf = np.arange(512)[None, :]
    in_maps = []
    for c in range(N_CORES):
        b, h = divmod(c, 2)
        gts = _slot_gtiles(h, causal)
        Xq = np.concatenate([encoded_q[b, g * P:(g + 1) * P, :] for g in gts], 0)
        xqh = np.ascontiguousarray(
            Xq.T.reshape(EO, P, 2, 512).transpose(2, 1, 0, 3))
        xkh = np.ascontiguousarray(
            encoded_k[b].T.reshape(EO, P, KC, 512).transpose(2, 1, 0, 3))
        xvh = np.ascontiguousarray(
            encoded_v[b].T.reshape(EO, P, 16, P).transpose(2, 1, 0, 3))
        cmh = np.zeros((P, 2, 512), np.float32)
        if causal:
            for p in range(2):
                r = 2 * h + p
                cmh[:, p, :] = np.where(kf <= r * P + qi, 0.0, NEG)
        in_maps.append({
            "xq": xqh.astype(np.float32), "xk": xkh.astype(np.float32),
            "xv": xvh.astype(np.float32),
            "wq": wqh, "wk": wkh, "wv": wvh,
            "bq": bqh, "bk": bkh, "bv": bvh, "cm": cmh,
        })
    return in_maps


def kernel(encoded_q, encoded_k, encoded_v, W_q, b_q, W_k, b_k, W_v, b_v,
           parameter_mask, _want_trace=False, _trace_dir=None):
    causal = bool(np.asarray(parameter_mask).item())
    encoded_q = np.asarray(encoded_q, np.float32)
    encoded_k = np.asarray(encoded_k, np.float32)
    encoded_v = np.asarray(encoded_v, np.float32)
    nc = _get_program(causal)
    in_maps = _shard_inputs(encoded_q, encoded_k, encoded_v,
                            np.asarray(W_q, np.float32), np.asarray(b_q, np.float32),
                            np.asarray(W_k, np.float32), np.asarray(b_k, np.float32),
                            np.asarray(W_v, np.float32), np.asarray(b_v, np.float32),
                            causal)
    kw = {}
    if _want_trace:
        kw = dict(trace=True, tmpdir=_trace_dir)
    res = run_bass_kernel_spmd(nc, in_maps, core_ids=list(range(N_CORES)), **kw)

    full = np.empty((B, S, D), np.float32)
    for c in range(N_CORES):
        b, h = divmod(c, 2)
        o = res.results[c]["out"]
        for j, g in enumerate(_slot_gtiles(h, causal)):
            full[b, g * P:(g + 1) * P, :] = o[j]
    if _want_trace:
        return full, res
    return full



# revision 3
# speedup vs baseline: 1.2939x; 1.2939x over previous
"""Trainium2 Bass kernel for single-head causal attention.

  q = Xq @ Wq.T + bq ; k = Xk @ Wk.T + bk ; v = Xv @ Wv.T + bv
  out = softmax((q k^T + causal_mask)/sqrt(D)) @ v

Shapes: B=4, S=2048, D=1024, fp32 in/out.  8 NeuronCores, SPMD.

Sharding: core c handles batch b = c//2, parity h = c%2.  S splits into 16
q-tiles of 128; causal attention for q-tile g touches k-tiles 0..g.  Core
parity h owns q-tiles g = 2j + h (j = 0..7), and slot j statically
processes 2j+2 k-tiles on BOTH parities (identical SPMD program); the
h=0 core's last k-tile per slot is fully masked, so the per-core causal
mask is one static [128, 256] tile covering the last two k-tiles.

Compute (all bf16 matmuls, f32 psum):
  - K^T, Q^T projected to [e-part, s] layout, V to [s-part, d]; all three
    stay fully SBUF-resident (no DRAM scratch roundtrip).
  - Scores are computed TRANSPOSED ([k, q] blocks): exp output feeds the
    P@V matmul directly as the stationary operand - no PE transposes.
  - softmax denominator = pe-block matmul against a ones column, giving
    [q-part, 1] psum, the right orientation for the final normalize
    (out = av * (1/den) + bv on DVE).
  - attention is software-pipelined one slot deep: scores(j+1) are issued
    before P@V(j) so the exp never stalls the tensor engine.
"""

from contextlib import ExitStack

import ml_dtypes
import numpy as np

import concourse.bacc as bacc
import concourse.mybir as mybir
import concourse.tile as tile
from concourse.bass_utils import run_bass_kernel_spmd

P = 128
D = 1024
S = 2048
B = 4
N_CORES = 8
EO = D // P            # 8 contraction chunks of 128
DO = D // P            # 8 output-dim chunks of 128
NT = S // P            # 16 k/s tiles of 128
NQ = 8                 # q-tile slots per core
F32 = mybir.dt.float32
BF16 = mybir.dt.bfloat16
NEG = -1.0e9
BF = ml_dtypes.bfloat16

_PROG_CACHE = {}


def _slot_gtiles(h, causal):
    """q-tile ids (units of 128 rows) owned by parity-h core, slot order."""
    if causal:
        return [2 * j + h for j in range(NQ)]
    return [8 * h + j for j in range(NQ)]


def build_program(causal: bool):
    nc = bacc.Bacc(trn_type="TRN2", target_bir_lowering=False, debug=False)

    def din(name, shape, dt=BF16):
        return nc.dram_tensor(name, shape, dt, kind="ExternalInput").ap()

    xq = din("xq", [P, EO, 1024])        # Xq^T for this core's 8 q-tiles
    xk = din("xk", [4, P, EO, 512])      # Xk^T, chunked along s
    xv = din("xv", [NT, P, EO, P])       # Xv^T, blocked [s-tile][e][s]
    wq = din("wq", [P, EO, D])
    wk = din("wk", [P, EO, D])
    wv = din("wv", [P, EO, D])
    bq = din("bq", [P, DO], F32)
    bk = din("bk", [P, DO], F32)
    bv = din("bv", [P, D], F32)
    msk = din("msk", [P, 2 * P], F32)    # causal mask for last 2 k-tiles
    out = nc.dram_tensor("out", [NQ, P, D], F32, kind="ExternalOutput").ap()

    Ident = mybir.ActivationFunctionType.Identity
    Exp = mybir.ActivationFunctionType.Exp
    add = mybir.AluOpType.add
    mult = mybir.AluOpType.mult

    # slot j processes nkt[j] k-tiles - identical on every core
    nkt = [2 * j + 2 if causal else NT for j in range(NQ)]

    with tile.TileContext(nc, pool_alloc_mode="queue") as tc, ExitStack() as top:
        const = top.enter_context(tc.tile_pool(name="const", bufs=1))
        bq_sb = const.tile([P, DO], F32)
        nc.gpsimd.dma_start(out=bq_sb, in_=bq)
        bk_sb = const.tile([P, DO], F32)
        nc.gpsimd.dma_start(out=bk_sb, in_=bk)
        bv_sb = const.tile([P, D], F32)
        nc.gpsimd.dma_start(out=bv_sb, in_=bv)
        msk_sb = const.tile([P, 2 * P], F32)
        nc.gpsimd.dma_start(out=msk_sb, in_=msk)
        ones_sb = const.tile([P, 1], BF16)
        nc.gpsimd.memset(ones_sb, 1.0)

        # resident projected tensors
        res = top.enter_context(tc.tile_pool(name="res", bufs=1))
        kt_sb = res.tile([P, DO, S], BF16, name="kt_sb")     # K^T [e, k]
        qt_sb = res.tile([P, DO, 1024], BF16, name="qt_sb")  # Q^T [e, q]
        v_sb = res.tile([P, NT, D], BF16, name="v_sb")       # V [s, d] blocked

        # ---------------- projections ----------------
        with tc.tile_pool(name="wt", bufs=2) as wtp, \
             tc.tile_pool(name="xin", bufs=2) as xinp, \
             tc.tile_pool(name="xqp", bufs=1) as xqp, \
             tc.tile_pool(name="xvp", bufs=2) as xvp, \
             tc.tile_pool(name="psA", bufs=3, space="PSUM") as psA, \
             tc.tile_pool(name="psB", bufs=2, space="PSUM") as psB:

            # K projection -> kt_sb [e-part, k], bias folded in
            wk_sb = wtp.tile([P, EO, D], BF16, tag="wt", name="wk_sb")
            for eo in range(EO):
                eng = nc.sync if eo % 2 == 0 else nc.scalar
                eng.dma_start(out=wk_sb[:, eo, :], in_=wk[:, eo, :])
            for kc in range(4):
                xk_t = xinp.tile([P, EO, 512], BF16, tag="xin", name=f"xk_t{kc}")
                for eo in range(EO):
                    eng = nc.sync if eo % 2 == 0 else nc.scalar
                    eng.dma_start(out=xk_t[:, eo, :], in_=xk[kc, :, eo, :])
                for do in range(DO):
                    ps = psA.tile([P, 512], F32, tag="psA", name=f"psk{kc}_{do}")
                    for eo in range(EO):
                        nc.tensor.matmul(
                            ps,
                            lhsT=wk_sb[:, eo, do * P:(do + 1) * P],
                            rhs=xk_t[:, eo, :],
                            start=(eo == 0), stop=(eo == EO - 1))
                    nc.scalar.activation(
                        out=kt_sb[:, do, kc * 512:(kc + 1) * 512], in_=ps,
                        func=Ident, bias=bk_sb[:, do:do + 1])

            # Q projection -> qt_sb [e-part, q], bias folded in
            wq_sb = wtp.tile([P, EO, D], BF16, tag="wt", name="wq_sb")
            xq_t = xqp.tile([P, EO, 1024], BF16, name="xq_t")
            for eo in range(EO):
                eng = nc.sync if eo % 2 == 0 else nc.scalar
                eng.dma_start(out=wq_sb[:, eo, :], in_=wq[:, eo, :])
                eng.dma_start(out=xq_t[:, eo, :], in_=xq[:, eo, :])
            for sc in range(2):
                for do in range(DO):
                    ps = psA.tile([P, 512], F32, tag="psA", name=f"psq{sc}_{do}")
                    for eo in range(EO):
                        nc.tensor.matmul(
                            ps,
                            lhsT=wq_sb[:, eo, do * P:(do + 1) * P],
                            rhs=xq_t[:, eo, sc * 512:(sc + 1) * 512],
                            start=(eo == 0), stop=(eo == EO - 1))
                    nc.scalar.activation(
                        out=qt_sb[:, do, sc * 512:(sc + 1) * 512], in_=ps,
                        func=Ident, bias=bq_sb[:, do:do + 1])

            # V projection -> v_sb [s-part, d] blocked; bias folded at output
            wv_sb = wtp.tile([P, EO, D], BF16, tag="wt", name="wv_sb")
            for eo in range(EO):
                eng = nc.sync if eo % 2 == 0 else nc.scalar
                eng.dma_start(out=wv_sb[:, eo, :], in_=wv[:, eo, :])
            for st in range(NT):
                xv_t = xvp.tile([P, EO, P], BF16, tag="xv", name=f"xv_t{st}")
                eng = nc.sync if st % 2 == 0 else nc.scalar
                eng.dma_start(out=xv_t, in_=xv[st])
                ps2 = psB.tile([P, D], F32, tag="psB", name=f"psv{st}")
                for half in range(2):
                    for eo in range(EO):
                        nc.tensor.matmul(
                            ps2[:, half * 512:(half + 1) * 512],
                            lhsT=xv_t[:, eo, :],
                            rhs=wv_sb[:, eo, half * 512:(half + 1) * 512],
                            start=(eo == 0), stop=(eo == EO - 1))
                nc.vector.tensor_copy(out=v_sb[:, st, :], in_=ps2)

        # ---------------- attention, q-slot-major, 1-slot pipeline ----------
        with tc.tile_pool(name="pep", bufs=2) as pep, \
             tc.tile_pool(name="recp", bufs=2) as recp, \
             tc.tile_pool(name="outp", bufs=2) as outp, \
             tc.tile_pool(name="psS", bufs=3, space="PSUM") as psS, \
             tc.tile_pool(name="psV", bufs=2, space="PSUM") as psV, \
             tc.tile_pool(name="psD", bufs=1, space="PSUM") as psD:

            dn_ps = psD.tile([P, NQ], F32, tag="dn", name="dn_ps")
            pes = [None] * NQ

            def scores_slot(j):
                n_t = nkt[j]
                qc = slice(j * P, (j + 1) * P)
                pe = pep.tile([P, n_t * P], BF16, tag="pe", name=f"pe{j}")
                pes[j] = pe
                for c in range((n_t + 3) // 4):
                    t0 = 4 * c
                    w = min(4, n_t - t0) * P
                    ps = psS.tile([P, w], F32, tag="s", name=f"ps{j}_{c}")
                    for tl in range(w // P):
                        for do in range(DO):
                            nc.tensor.matmul(
                                ps[:, tl * P:(tl + 1) * P],
                                lhsT=kt_sb[:, do, (t0 + tl) * P:(t0 + tl + 1) * P],
                                rhs=qt_sb[:, do, qc],
                                start=(do == 0), stop=(do == DO - 1))
                    if causal and t0 + w // P == n_t:
                        # mask covers the last two k-tiles of the slot
                        nc.vector.tensor_tensor(
                            out=ps[:, w - 2 * P:w], in0=ps[:, w - 2 * P:w],
                            in1=msk_sb, op=add)
                    nc.scalar.activation(
                        out=pe[:, t0 * P:t0 * P + w], in_=ps, func=Exp,
                        scale=float(1.0 / np.sqrt(D)))

            def av_slot(j):
                n_t = nkt[j]
                pe = pes[j]
                av = psV.tile([P, D], F32, tag="av", name=f"av{j}")
                for t in range(n_t):
                    pblk = pe[:, t * P:(t + 1) * P]
                    nc.tensor.matmul(
                        dn_ps[:, j:j + 1], lhsT=pblk, rhs=ones_sb,
                        start=(t == 0), stop=(t == n_t - 1))
                    for half in range(2):
                        nc.tensor.matmul(
                            av[:, half * 512:(half + 1) * 512],
                            lhsT=pblk,
                            rhs=v_sb[:, t, half * 512:(half + 1) * 512],
                            start=(t == 0), stop=(t == n_t - 1))
                rec = recp.tile([P, 1], F32, tag="rec", name=f"rec{j}")
                nc.vector.reciprocal(out=rec, in_=dn_ps[:, j:j + 1])
                o = outp.tile([P, D], F32, tag="o", name=f"o{j}")
                nc.vector.scalar_tensor_tensor(
                    out=o, in0=av, scalar=rec, in1=bv_sb,
                    op0=mult, op1=add)
                nc.gpsimd.dma_start(out=out[j], in_=o)

            for j in range(NQ):
                scores_slot(j)
                if j > 0:
                    av_slot(j - 1)
            av_slot(NQ - 1)

    nc.compile()
    return nc


def _get_program(causal: bool):
    key = bool(causal)
    if key not in _PROG_CACHE:
        _PROG_CACHE[key] = build_program(key)
    return _PROG_CACHE[key]


def _shard_inputs(encoded_q, encoded_k, encoded_v, W_q, b_q, W_k, b_k,
                  W_v, b_v, causal):
    """Build the per-core in_maps (all host-side numpy, bf16 payloads)."""
    wqh = np.ascontiguousarray(
        W_q.T.reshape(EO, P, D).transpose(1, 0, 2)).astype(BF)
    wkh = np.ascontiguousarray(
        W_k.T.reshape(EO, P, D).transpose(1, 0, 2)).astype(BF)
    wvh = np.ascontiguousarray(
        W_v.T.reshape(EO, P, D).transpose(1, 0, 2)).astype(BF)
    bqh = np.ascontiguousarray(b_q.reshape(DO, P).T)
    bkh = np.ascontiguousarray(b_k.reshape(DO, P).T)
    bvh = np.ascontiguousarray(np.broadcast_to(b_v, (P, D)))

    ki = np.arange(P)[:, None]
    qi = np.arange(P)[None, :]
    tri = np.where(ki <= qi, 0.0, NEG).astype(np.float32)   # diagonal block
    zer = np.zeros((P, P), np.float32)
    ninf = np.full((P, P), NEG, np.float32)
    # h=0: slot j owns g=2j -> k-tile 2j is diagonal, 2j+1 fully masked
    # h=1: slot j owns g=2j+1 -> k-tile 2j unmasked, 2j+1 diagonal
    mskh = [np.concatenate([tri, ninf], 1), np.concatenate([zer, tri], 1)]

    in_maps = []
    for c in range(N_CORES):
        b, h = divmod(c, 2)
        gts = _slot_gtiles(h, causal)
        Xq = np.concatenate([encoded_q[b, g * P:(g + 1) * P, :] for g in gts], 0)
        xqh = np.ascontiguousarray(
            Xq.T.reshape(EO, P, 1024).transpose(1, 0, 2)).astype(BF)
        xkh = np.ascontiguousarray(
            encoded_k[b].T.reshape(EO, P, 4, 512).transpose(2, 1, 0, 3)).astype(BF)
        xvh = np.ascontiguousarray(
            encoded_v[b].T.reshape(EO, P, NT, P).transpose(2, 1, 0, 3)).astype(BF)
        in_maps.append({
            "xq": xqh, "xk": xkh, "xv": xvh,
            "wq": wqh, "wk": wkh, "wv": wvh,
            "bq": bqh, "bk": bkh, "bv": bvh,
            "msk": mskh[h] if causal else np.zeros((P, 2 * P), np.float32),
        })
    return in_maps


def kernel(encoded_q, encoded_k, encoded_v, W_q, b_q, W_k, b_k, W_v, b_v,
           parameter_mask, _want_trace=False, _trace_dir=None):
    causal = bool(np.asarray(parameter_mask).item())
    encoded_q = np.asarray(encoded_q, np.float32)
    encoded_k = np.asarray(encoded_k, np.float32)
    encoded_v = np.asarray(encoded_v, np.float32)
    nc = _get_program(causal)
    in_maps = _shard_inputs(encoded_q, encoded_k, encoded_v,
                            np.asarray(W_q, np.float32), np.asarray(b_q, np.float32),
                            np.asarray(W_k, np.float32), np.asarray(b_k, np.float32),
                            np.asarray(W_v, np.float32), np.asarray(b_v, np.float32),
                            causal)
    kw = {}
    if _want_trace:
        kw = dict(trace=True, tmpdir=_trace_dir)
    res = run_bass_kernel_spmd(nc, in_maps, core_ids=list(range(N_CORES)), **kw)

    full = np.empty((B, S, D), np.float32)
    for c in range(N_CORES):
        b, h = divmod(c, 2)
        o = res.results[c]["out"]
        for j, g in enumerate(_slot_gtiles(h, causal)):
            full[b, g * P:(g + 1) * P, :] = o[j]
    if _want_trace:
        return full, res
    return full


# revision 9
# speedup vs baseline: 1.3169x; 1.0178x over previous
"""Trainium2 Bass kernel for single-head causal attention.

  q = Xq @ Wq.T + bq ; k = Xk @ Wk.T + bk ; v = Xv @ Wv.T + bv
  out = softmax((q k^T + causal_mask)/sqrt(D)) @ v

Shapes: B=4, S=2048, D=1024, fp32 in/out.  8 NeuronCores, SPMD.

Sharding: core c handles batch b = c//2, parity h = c%2.  S splits into 16
q-tiles of 128; causal attention for q-tile g touches k-tiles 0..g.  Core
parity h owns q-tiles g = 2j + h (j = 0..7), and slot j statically
processes 2j+2 k-tiles on BOTH parities (identical SPMD program); the
h=0 core's last k-tile per slot is fully masked, so the per-core causal
mask is one static [128, 256] tile covering the last two k-tiles.

Compute (all bf16 matmuls, f32 psum):
  - K^T, Q^T projected to [e-part, s] layout, V to [s-part, d]; all three
    stay fully SBUF-resident (no DRAM scratch roundtrip).
  - Scores are computed TRANSPOSED ([k, q] blocks): exp output feeds the
    P@V matmul directly as the stationary operand - no PE transposes.
  - softmax denominator = pe-block matmul against a ones column, giving
    [q-part, 1] psum, the right orientation for the final normalize
    (out = av * (1/den) + bv on DVE).
  - attention is software-pipelined one slot deep: scores(j+1) are issued
    before P@V(j) so the exp never stalls the tensor engine.
"""

from contextlib import ExitStack

import ml_dtypes
import numpy as np

import concourse.bacc as bacc
import concourse.mybir as mybir
import concourse.tile as tile
from concourse.bass_utils import run_bass_kernel_spmd

P = 128
D = 1024
S = 2048
B = 4
N_CORES = 8
EO = D // P            # 8 contraction chunks of 128
DO = D // P            # 8 output-dim chunks of 128
NT = S // P            # 16 k/s tiles of 128
NQ = 8                 # q-tile slots per core
F32 = mybir.dt.float32
BF16 = mybir.dt.bfloat16
NEG = -1.0e9
BF = ml_dtypes.bfloat16

_PROG_CACHE = {}


def _slot_gtiles(h, causal):
    """q-tile ids (units of 128 rows) owned by parity-h core, slot order."""
    if causal:
        return [2 * j + h for j in range(NQ)]
    return [8 * h + j for j in range(NQ)]


def build_program(causal: bool):
    nc = bacc.Bacc(trn_type="TRN2", target_bir_lowering=False, debug=False)

    def din(name, shape, dt=BF16):
        return nc.dram_tensor(name, shape, dt, kind="ExternalInput").ap()

    xq = din("xq", [P, EO, 1024])        # Xq^T for this core's 8 q-tiles
    xk = din("xk", [4, P, EO, 512])      # Xk^T, chunked along s
    xv = din("xv", [NT, P, EO, P])       # Xv^T, blocked [s-tile][e][s]
    wq = din("wq", [P, EO, D])
    wk = din("wk", [P, EO, D])
    wv = din("wv", [P, EO, D])
    bq = din("bq", [P, DO], F32)
    bk = din("bk", [P, DO], F32)
    bv = din("bv", [P, D], F32)
    msk = din("msk", [P, 2 * P], F32)    # causal mask for last 2 k-tiles
    out = nc.dram_tensor("out", [NQ, P, D], F32, kind="ExternalOutput").ap()

    Ident = mybir.ActivationFunctionType.Identity
    Exp = mybir.ActivationFunctionType.Exp
    add = mybir.AluOpType.add
    mult = mybir.AluOpType.mult

    # slot j processes nkt[j] k-tiles - identical on every core
    nkt = [2 * j + 2 if causal else NT for j in range(NQ)]

    with tile.TileContext(nc, pool_alloc_mode="queue") as tc, ExitStack() as top:
        const = top.enter_context(tc.tile_pool(name="const", bufs=1))
        bq_sb = const.tile([P, DO], F32)
        nc.gpsimd.dma_start(out=bq_sb, in_=bq)
        bk_sb = const.tile([P, DO], F32)
        nc.gpsimd.dma_start(out=bk_sb, in_=bk)
        bv_sb = const.tile([P, D], F32)
        nc.gpsimd.dma_start(out=bv_sb, in_=bv)
        msk_sb = const.tile([P, 2 * P], F32)
        nc.gpsimd.dma_start(out=msk_sb, in_=msk)
        ones_sb = const.tile([P, 1], BF16)
        nc.gpsimd.memset(ones_sb, 1.0)

        # resident projected tensors
        res = top.enter_context(tc.tile_pool(name="res", bufs=1))
        kt_sb = res.tile([P, DO, S], BF16, name="kt_sb")     # K^T [e, k]
        qt_sb = res.tile([P, DO, 1024], BF16, name="qt_sb")  # Q^T [e, q]
        v_sb = res.tile([P, NT, D], BF16, name="v_sb")       # V [s, d] blocked

        # ---------------- projections ----------------
        with tc.tile_pool(name="wt", bufs=2) as wtp, \
             tc.tile_pool(name="xin", bufs=3) as xinp, \
             tc.tile_pool(name="xqp", bufs=1) as xqp, \
             tc.tile_pool(name="xvp", bufs=4) as xvp, \
             tc.tile_pool(name="psA", bufs=3, space="PSUM") as psA, \
             tc.tile_pool(name="psB", bufs=2, space="PSUM") as psB:

            # K projection -> kt_sb [e-part, k], bias folded in
            # first-wave DMAs spread over 4 queues to cut startup latency
            qs = [nc.sync, nc.scalar, nc.gpsimd]
            wk_sb = wtp.tile([P, EO, D], BF16, tag="wt", name="wk_sb")
            xk_t0 = xinp.tile([P, EO, 512], BF16, tag="xin", name="xk_t0")
            for eo in range(EO):
                qs[eo % 3].dma_start(out=wk_sb[:, eo, :], in_=wk[:, eo, :])
                qs[(eo + 1) % 3].dma_start(out=xk_t0[:, eo, :], in_=xk[0, :, eo, :])
            for kc in range(4):
                if kc == 0:
                    xk_t = xk_t0
                else:
                    xk_t = xinp.tile([P, EO, 512], BF16, tag="xin",
                                     name=f"xk_t{kc}")
                    for eo in range(EO):
                        eng = nc.sync if eo % 2 == 0 else nc.scalar
                        eng.dma_start(out=xk_t[:, eo, :], in_=xk[kc, :, eo, :])
                for do in range(DO):
                    ps = psA.tile([P, 512], F32, tag="psA", name=f"psk{kc}_{do}")
                    for eo in range(EO):
                        nc.tensor.matmul(
                            ps,
                            lhsT=wk_sb[:, eo, do * P:(do + 1) * P],
                            rhs=xk_t[:, eo, :],
                            start=(eo == 0), stop=(eo == EO - 1))
                    nc.scalar.activation(
                        out=kt_sb[:, do, kc * 512:(kc + 1) * 512], in_=ps,
                        func=Ident, bias=bk_sb[:, do:do + 1])

            # Q projection -> qt_sb [e-part, q], bias folded in
            wq_sb = wtp.tile([P, EO, D], BF16, tag="wt", name="wq_sb")
            xq_t = xqp.tile([P, EO, 1024], BF16, name="xq_t")
            for eo in range(EO):
                eng = nc.sync if eo % 2 == 0 else nc.scalar
                eng.dma_start(out=wq_sb[:, eo, :], in_=wq[:, eo, :])
                eng.dma_start(out=xq_t[:, eo, :], in_=xq[:, eo, :])
            for sc in range(2):
                for do in range(DO):
                    ps = psA.tile([P, 512], F32, tag="psA", name=f"psq{sc}_{do}")
                    for eo in range(EO):
                        nc.tensor.matmul(
                            ps,
                            lhsT=wq_sb[:, eo, do * P:(do + 1) * P],
                            rhs=xq_t[:, eo, sc * 512:(sc + 1) * 512],
                            start=(eo == 0), stop=(eo == EO - 1))
                    nc.scalar.activation(
                        out=qt_sb[:, do, sc * 512:(sc + 1) * 512], in_=ps,
                        func=Ident, bias=bq_sb[:, do:do + 1])

            # V projection -> v_sb [s-part, d] blocked; bias folded at output
            wv_sb = wtp.tile([P, EO, D], BF16, tag="wt", name="wv_sb")
            for eo in range(EO):
                eng = nc.sync if eo % 2 == 0 else nc.scalar
                eng.dma_start(out=wv_sb[:, eo, :], in_=wv[:, eo, :])
            for st in range(NT):
                xv_t = xvp.tile([P, EO, P], BF16, tag="xv", name=f"xv_t{st}")
                eng = nc.sync if st % 2 == 0 else nc.gpsimd
                eng.dma_start(out=xv_t, in_=xv[st])
                ps2 = psB.tile([P, D], F32, tag="psB", name=f"psv{st}")
                for half in range(2):
                    for eo in range(EO):
                        nc.tensor.matmul(
                            ps2[:, half * 512:(half + 1) * 512],
                            lhsT=xv_t[:, eo, :],
                            rhs=wv_sb[:, eo, half * 512:(half + 1) * 512],
                            start=(eo == 0), stop=(eo == EO - 1))
                # evict on Act so DVE stays free for attention-phase work
                nc.scalar.activation(out=v_sb[:, st, :], in_=ps2,
                                     func=mybir.ActivationFunctionType.Copy)

        # ---------------- attention, q-slot-major, 1-slot pipeline ----------
        with tc.tile_pool(name="pep", bufs=2) as pep, \
             tc.tile_pool(name="recp", bufs=2) as recp, \
             tc.tile_pool(name="outp", bufs=2) as outp, \
             tc.tile_pool(name="psS", bufs=3, space="PSUM") as psS, \
             tc.tile_pool(name="psV", bufs=2, space="PSUM") as psV, \
             tc.tile_pool(name="psD", bufs=1, space="PSUM") as psD:

            dn_ps = psD.tile([P, NQ], F32, tag="dn", name="dn_ps")
            pes = [None] * NQ

            def scores_slot(j):
                n_t = nkt[j]
                qc = slice(j * P, (j + 1) * P)
                pe = pep.tile([P, n_t * P], BF16, tag="pe", name=f"pe{j}")
                pes[j] = pe
                for c in range((n_t + 3) // 4):
                    t0 = 4 * c
                    w = min(4, n_t - t0) * P
                    ps = psS.tile([P, w], F32, tag="s", name=f"ps{j}_{c}")
                    for tl in range(w // P):
                        for do in range(DO):
                            nc.tensor.matmul(
                                ps[:, tl * P:(tl + 1) * P],
                                lhsT=kt_sb[:, do, (t0 + tl) * P:(t0 + tl + 1) * P],
                                rhs=qt_sb[:, do, qc],
                                start=(do == 0), stop=(do == DO - 1))
                    if causal and t0 + w // P == n_t:
                        # mask covers the last two k-tiles of the slot
                        nc.vector.tensor_tensor(
                            out=ps[:, w - 2 * P:w], in0=ps[:, w - 2 * P:w],
                            in1=msk_sb, op=add)
                    nc.scalar.activation(
                        out=pe[:, t0 * P:t0 * P + w], in_=ps, func=Exp,
                        scale=float(1.0 / np.sqrt(D)))

            def av_slot(j):
                n_t = nkt[j]
                pe = pes[j]
                av = psV.tile([P, D], F32, tag="av", name=f"av{j}")
                for t in range(n_t):
                    pblk = pe[:, t * P:(t + 1) * P]
                    nc.tensor.matmul(
                        dn_ps[:, j:j + 1], lhsT=pblk, rhs=ones_sb,
                        start=(t == 0), stop=(t == n_t - 1))
                    for half in range(2):
                        nc.tensor.matmul(
                            av[:, half * 512:(half + 1) * 512],
                            lhsT=pblk,
                            rhs=v_sb[:, t, half * 512:(half + 1) * 512],
                            start=(t == 0), stop=(t == n_t - 1))
                rec = recp.tile([P, 1], F32, tag="rec", name=f"rec{j}")
                nc.vector.reciprocal(out=rec, in_=dn_ps[:, j:j + 1])
                o = outp.tile([P, D], F32, tag="o", name=f"o{j}")
                nc.vector.scalar_tensor_tensor(
                    out=o, in0=av, scalar=rec, in1=bv_sb,
                    op0=mult, op1=add)
                eng = nc.gpsimd if j % 2 == 0 else nc.sync
                eng.dma_start(out=out[j], in_=o)

            # big slots first: the kernel tail is the smallest slot's drain
            order = list(range(NQ - 1, -1, -1))
            for i, j in enumerate(order):
                scores_slot(j)
                if i > 0:
                    av_slot(order[i - 1])
            av_slot(order[-1])

    nc.compile()
    return nc


def _get_program(causal: bool):
    key = bool(causal)
    if key not in _PROG_CACHE:
        _PROG_CACHE[key] = build_program(key)
    return _PROG_CACHE[key]


def _shard_inputs(encoded_q, encoded_k, encoded_v, W_q, b_q, W_k, b_k,
                  W_v, b_v, causal):
    """Build the per-core in_maps (all host-side numpy, bf16 payloads)."""
    wqh = np.ascontiguousarray(
        W_q.T.reshape(EO, P, D).transpose(1, 0, 2)).astype(BF)
    wkh = np.ascontiguousarray(
        W_k.T.reshape(EO, P, D).transpose(1, 0, 2)).astype(BF)
    wvh = np.ascontiguousarray(
        W_v.T.reshape(EO, P, D).transpose(1, 0, 2)).astype(BF)
    bqh = np.ascontiguousarray(b_q.reshape(DO, P).T)
    bkh = np.ascontiguousarray(b_k.reshape(DO, P).T)
    bvh = np.ascontiguousarray(np.broadcast_to(b_v, (P, D)))

    ki = np.arange(P)[:, None]
    qi = np.arange(P)[None, :]
    tri = np.where(ki <= qi, 0.0, NEG).astype(np.float32)   # diagonal block
    zer = np.zeros((P, P), np.float32)
    ninf = np.full((P, P), NEG, np.float32)
    # h=0: slot j owns g=2j -> k-tile 2j is diagonal, 2j+1 fully masked
    # h=1: slot j owns g=2j+1 -> k-tile 2j unmasked, 2j+1 diagonal
    mskh = [np.concatenate([tri, ninf], 1), np.concatenate([zer, tri], 1)]

    in_maps = []
    for c in range(N_CORES):
        b, h = divmod(c, 2)
        gts = _slot_gtiles(h, causal)
        Xq = np.concatenate([encoded_q[b, g * P:(g + 1) * P, :] for g in gts], 0)
        xqh = np.ascontiguousarray(
            Xq.T.reshape(EO, P, 1024).transpose(1, 0, 2)).astype(BF)
        xkh = np.ascontiguousarray(
            encoded_k[b].T.reshape(EO, P, 4, 512).transpose(2, 1, 0, 3)).astype(BF)
        xvh = np.ascontiguousarray(
            encoded_v[b].T.reshape(EO, P, NT, P).transpose(2, 1, 0, 3)).astype(BF)
        in_maps.append({
            "xq": xqh, "xk": xkh, "xv": xvh,
            "wq": wqh, "wk": wkh, "wv": wvh,
            "bq": bqh, "bk": bkh, "bv": bvh,
            "msk": mskh[h] if causal else np.zeros((P, 2 * P), np.float32),
        })
    return in_maps


def kernel(encoded_q, encoded_k, encoded_v, W_q, b_q, W_k, b_k, W_v, b_v,
           parameter_mask, _want_trace=False, _trace_dir=None):
    causal = bool(np.asarray(parameter_mask).item())
    encoded_q = np.asarray(encoded_q, np.float32)
    encoded_k = np.asarray(encoded_k, np.float32)
    encoded_v = np.asarray(encoded_v, np.float32)
    nc = _get_program(causal)
    in_maps = _shard_inputs(encoded_q, encoded_k, encoded_v,
                            np.asarray(W_q, np.float32), np.asarray(b_q, np.float32),
                            np.asarray(W_k, np.float32), np.asarray(b_k, np.float32),
                            np.asarray(W_v, np.float32), np.asarray(b_v, np.float32),
                            causal)
    kw = {}
    if _want_trace:
        kw = dict(trace=True, tmpdir=_trace_dir)
    res = run_bass_kernel_spmd(nc, in_maps, core_ids=list(range(N_CORES)), **kw)

    full = np.empty((B, S, D), np.float32)
    for c in range(N_CORES):
        b, h = divmod(c, 2)
        o = res.results[c]["out"]
        for j, g in enumerate(_slot_gtiles(h, causal)):
            full[b, g * P:(g + 1) * P, :] = o[j]
    if _want_trace:
        return full, res
    return full
